# revision 4
# baseline (speedup 1.0000x reference)
"""LocalExpansion (7x7 unfold) Trainium2 Bass kernel, v2.

Full input x: [2, 8, 2304, 64] f32 (B=2, heads=8, N=48*48, D=64).
Full output:  [2, 8, 2304, 49, 64] f32 — out[b,h,y*W+x,i*7+j,:] =
x_img[b,h,y+i-3,x+j-3,:] with zero fill outside the 48x48 image.

Strategy (memory-regime, ~57.8 MB of HBM writes per core):
- batch*heads = 16 images, 2 per core across 8 NeuronCores.
- Partition map: image row y = 3g+r -> partition 4g+r (r in 0..2),
  image 0 in partitions 0-63, image 1 in 64-127. Skipping every 4th
  partition gives each of the 16 SDMA engines exactly 6 rows (the
  engine<->partition swizzle is quad-based), vs an 8/4 imbalance for
  a contiguous map. Skipped partitions stay zero; the fill op reads
  them but their U output is never stored.
- P[p] holds the 7 zero-padded image rows y(p)-3..y(p)+3 in rows 0-6
  plus an always-zero row 7 (8*54*64 floats). Init avoids a full-P
  memset (12.7 us) on the critical path: only row 3's pad columns and
  row 7 are memset (~2 us); replication copies full padded rows so
  rows 0-6 inherit row 3's pad zeros, and boundary rows (y+i-3
  outside the image) are zero-filled from row 7 by tiny same-partition
  SBUF->SBUF DMAs that run concurrently with replication. Skipped/dup
  partitions are left uninitialized; the fill reads junk there but
  that U output is never stored.
- The unfold is materialized in SBUF by the Vector engine
  (f32 tensor_copy, 2 elem/cycle/lane): U[p][x - x0][i*7+j][d] =
  P[p][i][x+j][d], double-buffered in x-tiles of XT=3 columns.
- Output DMA per (tile, image): src and dst are both contiguous runs
  of 3*49*64 floats (37632 B descriptors, one per partition, written
  in ascending address order -> sequential HBM write streams), vs
  1792 B scattered descriptors for the direct sliding-window DMA.
- Image 0 DMAs ride the sync HWDGE ring / even SDMA engines; image 1
  rides the scalar ring / odd engines.
"""

import numpy as np

KH, KW = 7, 7
H, W, D = 48, 48, 64
K = KH * KW                 # 49
N = H * W                   # 2304
PR = (W + 6) * D            # 3456 floats per padded row
NR = (KH + 1) * PR          # 27648 floats of P per partition (row 7 = zeros)
XT = 3                      # x columns per tile
NT = W // XT                # 16 tiles
CH = XT * K * D             # 9408 floats per out chunk (37632 B)
FU = 2 * CH                 # 18816 floats of U per partition (2 bufs)
RB = W * K * D              # 150528 floats per output row
ND = N * D                  # floats per image input
NKD = N * K * D             # floats per image output
IMGS_PER_CORE = 2
N_CORES = 8

_CACHE = {}


def _rep_dmas():
    """Replication plan: (dst_part, src_part, row_i, count) per DMA.

    dst partition 4g+r (+64*im) row i=delta+3 <- row 3 of the partition
    holding row y' = 3g + r + delta, for g in an affine range (rows with
    y' outside [0,48) keep the memset zeros).
    """
    plan = []
    for r in range(3):
        for delta in (-3, -2, -1, 1, 2, 3):
            s = r + delta
            q = s // 3
            rr = s - 3 * q
            g_lo = max(0, -q)
            g_hi = min(15, 15 - q)
            if g_hi < g_lo:
                continue
            plan.append((4 * g_lo + r, 4 * (g_lo + q) + rr, delta + 3,
                         g_hi - g_lo + 1))
    return plan


def _build_nc():
    import concourse.bass as bass
    import concourse.mybir as mybir

    nc = bass.Bass(trn_type="TRN2")
    x = nc.dram_tensor("x", [IMGS_PER_CORE, N, D], mybir.dt.float32,
                       kind="ExternalInput")
    out = nc.dram_tensor("out", [IMGS_PER_CORE, N, K, D], mybir.dt.float32,
                         kind="ExternalOutput")

    rep = _rep_dmas()
    n_rep = 2 * len(rep)

    with (
        nc.sbuf_tensor("P", [128, NR], mybir.dt.float32) as P,
        nc.sbuf_tensor("U", [128, FU], mybir.dt.float32) as U,
        nc.semaphore("ms") as ms,    # memsets (row-3 pads, zero row 7)
        nc.semaphore("ld") as ld,    # input loads
        nc.semaphore("rp") as rp,    # replication DMAs
        nc.semaphore("zn") as zn,    # boundary-row zero fills
        nc.semaphore("fl") as fl,    # vector fills
        nc.semaphore("st0") as st0,  # out DMAs, sync ring (image 0)
        nc.semaphore("st1") as st1,  # out DMAs, scalar ring (image 1)
    ):
        # Memsets: row 3's pad columns (replication propagates them to
        # rows 0-6) and the always-zero row 7.
        nc.vector.memset(
            bass.AP(P, 3 * PR, [[NR, 128], [1, 3 * D]]), 0.0
        ).then_inc(ms, 1)
        nc.vector.memset(
            bass.AP(P, 3 * PR + 3 * D + W * D, [[NR, 128], [1, 3 * D]]), 0.0
        ).then_inc(ms, 1)
        nc.vector.memset(
            bass.AP(P, KH * PR, [[NR, 128], [1, PR]]), 0.0
        ).then_inc(ms, 1)

        # Loads: row y=3g+r of image im -> partition 64*im+4g+r, row 3
        # interior. One DMA per (image, r): only the FIRST dim of an
        # SBUF-side DMA AP may step partitions on HW. All P-writing DMAs
        # ride the sync ring so they are FIFO-ordered on one queue (the
        # engines still spread work by partition); out DMAs use both.
        for im, ring in ((0, nc.sync), (1, nc.scalar)):
            for r in range(3):
                ring.dma_start(
                    out=bass.AP(P, (64 * im + r) * NR + 3 * PR + 3 * D,
                                [[4 * NR, 16], [1, W * D]]),
                    in_=bass.AP(x, im * ND + r * W * D,
                                [[3 * W * D, 16], [1, W * D]]),
                ).then_inc(ld, 16)
        n_ld = 6

        # Boundary rows (y+i-3 outside [0,48)) zero-filled from row 7 of
        # the same partition. (part_base, row, count) over valid rows
        # only (dup partitions are never stored). Issued before the ld
        # wait so they drain during the loads (disjoint bytes).
        zones = [(0, 0, 3), (0, 1, 2), (0, 2, 1),
                 (60, 6, 3), (61, 5, 2), (62, 4, 1)]
        for im, ring in ((0, nc.sync), (1, nc.scalar)):
            ring.wait_ge(ms, 3)
            for pb, row, cnt in zones:
                ring.dma_start(
                    out=bass.AP(P, (64 * im + pb) * NR + row * PR,
                                [[NR, cnt], [1, PR]]),
                    in_=bass.AP(P, (64 * im + pb) * NR + KH * PR,
                                [[NR, cnt], [1, PR]]),
                ).then_inc(zn, 16)
        n_zones = 12

        # Replication: P[p][i] <- P[p'][3] (full padded rows, same half).
        # Per-image ring split: the ~1.5us/DMA serialization is per ring.
        for im, ring in ((0, nc.sync), (1, nc.scalar)):
            ring.wait_ge(ld, n_ld * 16)
            for dp, sp, row_i, cnt in rep:
                ring.dma_start(
                    out=bass.AP(P, (64 * im + dp) * NR + row_i * PR,
                                [[4 * NR, cnt], [1, PR]]),
                    in_=bass.AP(P, (64 * im + sp) * NR + 3 * PR,
                                [[4 * NR, cnt], [1, PR]]),
                ).then_inc(rp, 16)

        # Fills, double-buffered in x-tiles: U[p][x-x0][k][d] =
        # P[p][i][x+j][d] for k = i*7+j. The strided copy runs at 1x DVE
        # mode (~9.9us/tile for all 7 i-rows), which would tie the out
        # DMAs (~10.1us/tile); split i-rows 0-3 on vector and 4-6 on
        # gpsimd so the fill stays off the critical path.
        splits = ((nc.vector, 0, 4), (nc.gpsimd, 4, 3))
        fpt = len(splits)
        for eng, _, _ in splits:
            eng.wait_ge(rp, n_rep * 16)
            eng.wait_ge(zn, n_zones * 16)
        for t in range(NT):
            buf = t % 2
            for eng, i0, ni in splits:
                if t >= 2:
                    eng.wait_ge(st0, 48 * (t - 1))
                    eng.wait_ge(st1, 48 * (t - 1))
                eng.tensor_copy(
                    bass.AP(U, buf * CH + i0 * KW * D,
                            [[FU, 127], [K * D, XT], [KW * D, ni], [1, KW * D]]),
                    bass.AP(P, t * XT * D + i0 * PR,
                            [[NR, 127], [D, XT], [PR, ni], [1, KW * D]]),
                ).then_inc(fl, 1)

        # Out DMAs: per (tile, image, r) one DMA, 16 descriptors of
        # XT*49*64 floats, ascending dst addresses.
        for t in range(NT):
            buf = t % 2
            for im, ring, sem in ((0, nc.sync, st0), (1, nc.scalar, st1)):
                ring.wait_ge(fl, fpt * (t + 1))
                for r in range(3):
                    ring.dma_start(
                        out=bass.AP(out, im * NKD + r * RB + t * CH,
                                    [[3 * RB, 16], [1, CH]]),
                        in_=bass.AP(U, (64 * im + r) * FU + buf * CH,
                                    [[4 * FU, 16], [1, CH]]),
                    ).then_inc(sem, 16)

        nc.sync.wait_ge(st0, NT * 48)
        nc.sync.wait_ge(st1, NT * 48)
        nc.scalar.wait_ge(st0, NT * 48)
        nc.scalar.wait_ge(st1, NT * 48)
    return nc


def kernel(x, height=48, width=48):
    from concourse.bass_utils import run_bass_kernel_spmd

    x = np.asarray(x)
    b, nh = x.shape[0], x.shape[1]
    xi = np.ascontiguousarray(x.reshape(b * nh, N, D))
    in_maps = [
        {"x": np.ascontiguousarray(xi[IMGS_PER_CORE * c: IMGS_PER_CORE * (c + 1)])}
        for c in range(N_CORES)
    ]
    if "nc" not in _CACHE:
        _CACHE["nc"] = _build_nc()
    res = run_bass_kernel_spmd(_CACHE["nc"], in_maps, core_ids=list(range(N_CORES)))
    y = np.stack([res.results[c]["out"] for c in range(N_CORES)])
    return y.reshape(b, nh, N, K, D).astype(np.float32, copy=False)


# revision 5
# speedup vs baseline: 1.3317x; 1.3317x over previous
"""LocalExpansion (7x7 unfold) Trainium2 Bass kernel, v2.

Full input x: [2, 8, 2304, 64] f32 (B=2, heads=8, N=48*48, D=64).
Full output:  [2, 8, 2304, 49, 64] f32 — out[b,h,y*W+x,i*7+j,:] =
x_img[b,h,y+i-3,x+j-3,:] with zero fill outside the 48x48 image.

Strategy (memory-regime, ~57.8 MB of HBM writes per core):
- batch*heads = 16 images, 2 per core across 8 NeuronCores.
- Partition map: image row y = 3g+r -> partition 4g+r (r in 0..2),
  image 0 in partitions 0-63, image 1 in 64-127. Skipping every 4th
  partition gives each of the 16 SDMA engines exactly 6 rows (the
  engine<->partition swizzle is quad-based), vs an 8/4 imbalance for
  a contiguous map. Skipped partitions stay zero; the fill op reads
  them but their U output is never stored.
- P[p] holds the 7 zero-padded image rows y(p)-3..y(p)+3 (7*54*64
  floats). Init avoids a full-P memset (12.7 us) on the critical
  path: only row 3's pad columns are memset (~0.4 us); replication
  copies full padded rows so rows 0-6 inherit row 3's pad zeros, and
  boundary rows (y+i-3 outside the image) are zeroed by scalar-engine
  memzero ops that overlap the replication drain. Skipped/dup
  partitions are left uninitialized; the fill reads junk there but
  that U output is never stored.
- The unfold is materialized in SBUF, double-buffered in x-tiles of
  XT=3 columns: U[p][x - x0][i*7+j][d] = P[p][i][x+j][d]. The strided
  copy runs at 1x mode, so it is split: i-rows 0-3 on the Vector
  engine, 4-6 on the Scalar (ACT) engine.
- Output DMA per (tile, image): src and dst are both contiguous runs
  of 3*49*64 floats (37632 B descriptors, one per partition, written
  in ascending address order -> sequential HBM write streams), vs
  1792 B scattered descriptors for the direct sliding-window DMA.
- Image 0 DMAs ride the sync HWDGE ring / even SDMA engines; image 1
  rides the scalar ring / odd engines.
"""

import numpy as np

KH, KW = 7, 7
H, W, D = 48, 48, 64
K = KH * KW                 # 49
N = H * W                   # 2304
PR = (W + 6) * D            # 3456 floats per padded row
NR = KH * PR                # 24192 floats of P per partition
XT = 3                      # x columns per tile
NT = W // XT                # 16 tiles
CH = XT * K * D             # 9408 floats per out chunk (37632 B)
FU = 2 * CH                 # 18816 floats of U per partition (2 bufs)
RB = W * K * D              # 150528 floats per output row
ND = N * D                  # floats per image input
NKD = N * K * D             # floats per image output
IMGS_PER_CORE = 2
N_CORES = 8

_CACHE = {}


def _rep_dmas():
    """Replication plan: (dst_part, src_part, row_i, count) per DMA.

    dst partition 4g+r (+64*im) row i=delta+3 <- row 3 of the partition
    holding row y' = 3g + r + delta, for g in an affine range (rows with
    y' outside [0,48) keep the memset zeros).
    """
    plan = []
    for r in range(3):
        for delta in (-3, -2, -1, 1, 2, 3):
            s = r + delta
            q = s // 3
            rr = s - 3 * q
            g_lo = max(0, -q)
            g_hi = min(15, 15 - q)
            if g_hi < g_lo:
                continue
            plan.append((4 * g_lo + r, 4 * (g_lo + q) + rr, delta + 3,
                         g_hi - g_lo + 1))
    return plan


def _build_nc():
    import concourse.bass as bass
    import concourse.mybir as mybir

    nc = bass.Bass(trn_type="TRN2")
    x = nc.dram_tensor("x", [IMGS_PER_CORE, N, D], mybir.dt.float32,
                       kind="ExternalInput")
    out = nc.dram_tensor("out", [IMGS_PER_CORE, N, K, D], mybir.dt.float32,
                         kind="ExternalOutput")

    rep = _rep_dmas()
    n_rep = 2 * len(rep)

    with (
        nc.sbuf_tensor("P", [128, NR], mybir.dt.float32) as P,
        nc.sbuf_tensor("U", [128, FU], mybir.dt.float32) as U,
        nc.semaphore("ms") as ms,    # memsets (row-3 pads, zero row 7)
        nc.semaphore("ld") as ld,    # input loads
        nc.semaphore("rp") as rp,    # replication DMAs
        nc.semaphore("zn") as zn,    # boundary-row zero fills
        nc.semaphore("fl") as fl,    # vector fills
        nc.semaphore("st0") as st0,  # out DMAs, sync ring (image 0)
        nc.semaphore("st1") as st1,  # out DMAs, scalar ring (image 1)
    ):
        # Memsets: row 3's pad columns (replication propagates them to
        # rows 0-6).
        nc.vector.memset(
            bass.AP(P, 3 * PR, [[NR, 128], [1, 3 * D]]), 0.0
        ).then_inc(ms, 1)
        nc.vector.memset(
            bass.AP(P, 3 * PR + 3 * D + W * D, [[NR, 128], [1, 3 * D]]), 0.0
        ).then_inc(ms, 1)

        # Loads: row y=3g+r of image im -> partition 64*im+4g+r, row 3
        # interior. One DMA per (image, r): only the FIRST dim of an
        # SBUF-side DMA AP may step partitions on HW. All P-writing DMAs
        # ride the sync ring so they are FIFO-ordered on one queue (the
        # engines still spread work by partition); out DMAs use both.
        for im, ring in ((0, nc.sync), (1, nc.scalar)):
            for r in range(3):
                ring.dma_start(
                    out=bass.AP(P, (64 * im + r) * NR + 3 * PR + 3 * D,
                                [[4 * NR, 16], [1, W * D]]),
                    in_=bass.AP(x, im * ND + r * W * D,
                                [[3 * W * D, 16], [1, W * D]]),
                ).then_inc(ld, 16)
        n_ld = 6

        # Boundary-row zeros. Engine SBUF ops must start at partition
        # 0/32/64/96, so the bottom rows (partitions 60-62) are covered
        # by an over-wide memset of rows 4-6 on partitions 32-62; the
        # delta>0 replication phase rewrites the valid ones afterwards.
        # Bottom overs first (they gate replication phase B), then the
        # exact top zones (gate fills only).
        for im in (0, 1):  # zn +1 each: rows 4-6, partitions 32-62
            nc.vector.memset(
                bass.AP(P, (64 * im + 32) * NR + 4 * PR,
                        [[NR, 31], [1, 3 * PR]]), 0.0
            ).then_inc(zn, 1)
        top = [(0, 0, 3), (0, 1, 2), (0, 2, 1)]
        for im in (0, 1):  # zn +1 each
            for pb, row, cnt in top:
                nc.vector.memset(
                    bass.AP(P, (64 * im + pb) * NR + row * PR,
                            [[NR, cnt], [1, PR]]), 0.0
                ).then_inc(zn, 1)
        n_zones = 8

        # Replication: P[p][i] <- P[p'][3] (full padded rows, same half).
        # Per-image ring split: the ~1.5us/DMA serialization is per ring.
        # Phase A (delta<0, rows 0-2) runs as soon as loads land; phase B
        # (delta>0, rows 4-6) additionally waits for the bottom overs it
        # must overwrite.
        rep_a = [r for r in rep if r[2] < 3]
        rep_b = [r for r in rep if r[2] > 3]
        for im, ring in ((0, nc.sync), (1, nc.scalar)):
            ring.wait_ge(ld, n_ld * 16)
            ring.wait_ge(ms, 2)
            for phase, dmas in enumerate((rep_a, rep_b)):
                if phase == 1:
                    ring.wait_ge(zn, 2)
                for dp, sp, row_i, cnt in dmas:
                    ring.dma_start(
                        out=bass.AP(P, (64 * im + dp) * NR + row_i * PR,
                                    [[4 * NR, cnt], [1, PR]]),
                        in_=bass.AP(P, (64 * im + sp) * NR + 3 * PR,
                                    [[4 * NR, cnt], [1, PR]]),
                    ).then_inc(rp, 16)

        # Fills, double-buffered in x-tiles: U[p][x-x0][k][d] =
        # P[p][i][x+j][d] for k = i*7+j. The strided copy runs at 1x DVE
        # mode (~9.9us/tile for all 7 i-rows), which would tie the out
        # DMAs (~10.1us/tile); split i-rows 0-3 on vector and 4-6 on the
        # scalar (ACT) engine so the fill stays off the critical path.
        # (GpSimd takes 15us for the 3-row part - too slow.)
        splits = ((nc.vector, 0, 4), (nc.scalar, 4, 3))
        fpt = len(splits)
        for eng, _, _ in splits:
            eng.wait_ge(rp, n_rep * 16)
            eng.wait_ge(zn, n_zones)

        def fill(eng, t, i0, ni):
            buf = t % 2
            if t >= 2:
                eng.wait_ge(st0, 48 * (t - 1))
                eng.wait_ge(st1, 48 * (t - 1))
            dst = bass.AP(U, buf * CH + i0 * KW * D,
                          [[FU, 127], [K * D, XT], [KW * D, ni], [1, KW * D]])
            src = bass.AP(P, t * XT * D + i0 * PR,
                          [[NR, 127], [D, XT], [PR, ni], [1, KW * D]])
            op = eng.tensor_copy(dst, src) if eng is nc.vector \
                else eng.copy(dst, src)
            op.then_inc(fl, 1)

        def store(t, im, ring, sem):
            buf = t % 2
            ring.wait_ge(fl, fpt * (t + 1))
            for r in range(3):
                ring.dma_start(
                    out=bass.AP(out, im * NKD + r * RB + t * CH,
                                [[3 * RB, 16], [1, CH]]),
                    in_=bass.AP(U, (64 * im + r) * FU + buf * CH,
                                [[4 * FU, 16], [1, CH]]),
                ).then_inc(sem, 16)

        # Vector: pure fill stream. Scalar: fill-part then its ring's
        # out-DMA issues, interleaved per tile (all-fills-then-all-
        # stores would deadlock the scalar sequencer on st1). Sync
        # ring: pure store stream.
        for t in range(NT):
            fill(nc.vector, t, 0, 4)
        for t in range(NT):
            fill(nc.scalar, t, 4, 3)
            store(t, 1, nc.scalar, st1)
        for t in range(NT):
            store(t, 0, nc.sync, st0)

        nc.sync.wait_ge(st0, NT * 48)
        nc.sync.wait_ge(st1, NT * 48)
        nc.scalar.wait_ge(st0, NT * 48)
        nc.scalar.wait_ge(st1, NT * 48)
    return nc


def kernel(x, height=48, width=48):
    from concourse.bass_utils import run_bass_kernel_spmd

    x = np.asarray(x)
    b, nh = x.shape[0], x.shape[1]
    xi = np.ascontiguousarray(x.reshape(b * nh, N, D))
    in_maps = [
        {"x": np.ascontiguousarray(xi[IMGS_PER_CORE * c: IMGS_PER_CORE * (c + 1)])}
        for c in range(N_CORES)
    ]
    if "nc" not in _CACHE:
        _CACHE["nc"] = _build_nc()
    res = run_bass_kernel_spmd(_CACHE["nc"], in_maps, core_ids=list(range(N_CORES)))
    y = np.stack([res.results[c]["out"] for c in range(N_CORES)])
    return y.reshape(b, nh, N, K, D).astype(np.float32, copy=False)


# revision 6
# speedup vs baseline: 1.4074x; 1.0568x over previous
"""LocalExpansion (7x7 unfold) Trainium2 Bass kernel, v2.

Full input x: [2, 8, 2304, 64] f32 (B=2, heads=8, N=48*48, D=64).
Full output:  [2, 8, 2304, 49, 64] f32 — out[b,h,y*W+x,i*7+j,:] =
x_img[b,h,y+i-3,x+j-3,:] with zero fill outside the 48x48 image.

Strategy (memory-regime, ~57.8 MB of HBM writes per core):
- batch*heads = 16 images, 2 per core across 8 NeuronCores.
- Partition map: image row y = 3g+r -> partition 4g+r (r in 0..2),
  image 0 in partitions 0-63, image 1 in 64-127. Skipping every 4th
  partition gives each of the 16 SDMA engines exactly 6 rows (the
  engine<->partition swizzle is quad-based), vs an 8/4 imbalance for
  a contiguous map. Skipped partitions stay zero; the fill op reads
  them but their U output is never stored.
- P[p] holds the 7 zero-padded image rows y(p)-3..y(p)+3 (7*54*64
  floats). Init avoids a full-P memset (12.7 us) on the critical
  path: only row 3's pad columns are memset (~0.4 us); replication
  copies full padded rows so rows 0-6 inherit row 3's pad zeros, and
  boundary rows (y+i-3 outside the image) are zeroed by scalar-engine
  memzero ops that overlap the replication drain. Skipped/dup
  partitions are left uninitialized; the fill reads junk there but
  that U output is never stored.
- The unfold is materialized in SBUF, double-buffered in x-tiles of
  XT=3 columns: U[p][x - x0][i*7+j][d] = P[p][i][x+j][d]. The strided
  copy runs at 1x mode, so it is split: i-rows 0-3 on the Vector
  engine, 4-6 on the Scalar (ACT) engine.
- Output DMA per (tile, image): src and dst are both contiguous runs
  of 3*49*64 floats (37632 B descriptors, one per partition, written
  in ascending address order -> sequential HBM write streams), vs
  1792 B scattered descriptors for the direct sliding-window DMA.
- Image 0 DMAs ride the sync HWDGE ring / even SDMA engines; image 1
  rides the scalar ring / odd engines.
"""

import numpy as np

KH, KW = 7, 7
H, W, D = 48, 48, 64
K = KH * KW                 # 49
N = H * W                   # 2304
PR = (W + 6) * D            # 3456 floats per padded row
NR = (KH + 1) * PR          # 27648 floats of P per partition (row 7 = zeros)
XT = 3                      # x columns per tile
NT = W // XT                # 16 tiles
CH = XT * K * D             # 9408 floats per out chunk (37632 B)
FU = 2 * CH                 # 18816 floats of U per partition (2 bufs)
RB = W * K * D              # 150528 floats per output row
ND = N * D                  # floats per image input
NKD = N * K * D             # floats per image output
IMGS_PER_CORE = 2
N_CORES = 8

_CACHE = {}


def _rep_dmas():
    """Replication plan: (dst_part, src_part, row_i, count) per DMA.

    dst partition 4g+r (+64*im) row i=delta+3 <- row 3 of the partition
    holding row y' = 3g + r + delta, for g in an affine range (rows with
    y' outside [0,48) keep the memset zeros).
    """
    plan = []
    for r in range(3):
        for delta in (-3, -2, -1, 1, 2, 3):
            s = r + delta
            q = s // 3
            rr = s - 3 * q
            g_lo = max(0, -q)
            g_hi = min(15, 15 - q)
            if g_hi < g_lo:
                continue
            plan.append((4 * g_lo + r, 4 * (g_lo + q) + rr, delta + 3,
                         g_hi - g_lo + 1))
    return plan


def _build_nc():
    import concourse.bass as bass
    import concourse.mybir as mybir

    nc = bass.Bass(trn_type="TRN2")
    x = nc.dram_tensor("x", [IMGS_PER_CORE, N, D], mybir.dt.float32,
                       kind="ExternalInput")
    out = nc.dram_tensor("out", [IMGS_PER_CORE, N, K, D], mybir.dt.float32,
                         kind="ExternalOutput")

    rep = _rep_dmas()
    n_rep = 2 * len(rep)

    with (
        nc.sbuf_tensor("P", [128, NR], mybir.dt.float32) as P,
        nc.sbuf_tensor("U", [128, FU], mybir.dt.float32) as U,
        nc.semaphore("ms") as ms,    # memsets (row-3 pads, zero row 7)
        nc.semaphore("ld") as ld,    # input loads
        nc.semaphore("rp") as rp,    # replication DMAs
        nc.semaphore("zn") as zn,    # boundary-row zero fills
        nc.semaphore("fl") as fl,    # vector fills
        nc.semaphore("st0") as st0,  # out DMAs, sync ring (image 0)
        nc.semaphore("st1") as st1,  # out DMAs, scalar ring (image 1)
    ):
        # Memsets: row 3's pad columns (replication propagates them to
        # rows 0-6) and the always-zero row 7 (zone source).
        nc.vector.memset(
            bass.AP(P, 3 * PR, [[NR, 128], [1, 3 * D]]), 0.0
        ).then_inc(ms, 1)
        nc.vector.memset(
            bass.AP(P, 3 * PR + 3 * D + W * D, [[NR, 128], [1, 3 * D]]), 0.0
        ).then_inc(ms, 1)
        nc.vector.memset(
            bass.AP(P, KH * PR, [[NR, 128], [1, PR]]), 0.0
        ).then_inc(ms, 1)

        # Loads: row y=3g+r of image im -> partition 64*im+4g+r, row 3
        # interior. One DMA per (image, r): only the FIRST dim of an
        # SBUF-side DMA AP may step partitions on HW. All P-writing DMAs
        # ride the sync ring so they are FIFO-ordered on one queue (the
        # engines still spread work by partition); out DMAs use both.
        for im, ring in ((0, nc.sync), (1, nc.scalar)):
            for r in range(3):
                ring.dma_start(
                    out=bass.AP(P, (64 * im + r) * NR + 3 * PR + 3 * D,
                                [[4 * NR, 16], [1, W * D]]),
                    in_=bass.AP(x, im * ND + r * W * D,
                                [[3 * W * D, 16], [1, W * D]]),
                ).then_inc(ld, 16)
        n_ld = 6

        # Boundary rows (y+i-3 outside [0,48)) zero-filled from row 7 of
        # the same partition, on the gpsimd (SWDGE) ring so they run in
        # parallel with the replication on the two HWDGE rings. They
        # only gate the fills (zn), not the replication.
        zones = [(0, 0, 3), (0, 1, 2), (0, 2, 1),
                 (60, 6, 3), (61, 5, 2), (62, 4, 1)]
        nc.gpsimd.wait_ge(ms, 3)
        for im in (0, 1):
            for pb, row, cnt in zones:
                nc.gpsimd.dma_start(
                    out=bass.AP(P, (64 * im + pb) * NR + row * PR,
                                [[NR, cnt], [1, PR]]),
                    in_=bass.AP(P, (64 * im + pb) * NR + KH * PR,
                                [[NR, cnt], [1, PR]]),
                ).then_inc(zn, 16)
        n_zones = 12 * 16

        # Replication: P[p][i] <- P[p'][3] (full padded rows, same half).
        # Per-image ring split: the ~1.5us/DMA serialization is per ring.
        for im, ring in ((0, nc.sync), (1, nc.scalar)):
            ring.wait_ge(ld, n_ld * 16)
            ring.wait_ge(ms, 2)
            for dp, sp, row_i, cnt in rep:
                ring.dma_start(
                    out=bass.AP(P, (64 * im + dp) * NR + row_i * PR,
                                [[4 * NR, cnt], [1, PR]]),
                    in_=bass.AP(P, (64 * im + sp) * NR + 3 * PR,
                                [[4 * NR, cnt], [1, PR]]),
                ).then_inc(rp, 16)

        # Fills, double-buffered in x-tiles: U[p][x-x0][k][d] =
        # P[p][i][x+j][d] for k = i*7+j. The strided copy runs at 1x DVE
        # mode (~9.9us/tile for all 7 i-rows), which would tie the out
        # DMAs (~10.1us/tile); split i-rows 0-3 on vector and 4-6 on the
        # scalar (ACT) engine so the fill stays off the critical path.
        # (GpSimd takes 15us for the 3-row part - too slow.)
        splits = ((nc.vector, 0, 4), (nc.scalar, 4, 3))
        fpt = len(splits)
        for eng, _, _ in splits:
            eng.wait_ge(rp, n_rep * 16)
            eng.wait_ge(zn, n_zones)

        def fill(eng, t, i0, ni):
            buf = t % 2
            if t >= 2:
                eng.wait_ge(st0, 48 * (t - 1))
                eng.wait_ge(st1, 48 * (t - 1))
            dst = bass.AP(U, buf * CH + i0 * KW * D,
                          [[FU, 127], [K * D, XT], [KW * D, ni], [1, KW * D]])
            src = bass.AP(P, t * XT * D + i0 * PR,
                          [[NR, 127], [D, XT], [PR, ni], [1, KW * D]])
            op = eng.tensor_copy(dst, src) if eng is nc.vector \
                else eng.copy(dst, src)
            op.then_inc(fl, 1)

        def store(t, im, ring, sem):
            buf = t % 2
            ring.wait_ge(fl, fpt * (t + 1))
            for r in range(3):
                ring.dma_start(
                    out=bass.AP(out, im * NKD + r * RB + t * CH,
                                [[3 * RB, 16], [1, CH]]),
                    in_=bass.AP(U, (64 * im + r) * FU + buf * CH,
                                [[4 * FU, 16], [1, CH]]),
                ).then_inc(sem, 16)

        # Vector: pure fill stream. Scalar: fill-part then its ring's
        # out-DMA issues, interleaved per tile (all-fills-then-all-
        # stores would deadlock the scalar sequencer on st1). Sync
        # ring: pure store stream.
        for t in range(NT):
            fill(nc.vector, t, 0, 4)
        for t in range(NT):
            fill(nc.scalar, t, 4, 3)
            store(t, 1, nc.scalar, st1)
        for t in range(NT):
            store(t, 0, nc.sync, st0)

        nc.sync.wait_ge(st0, NT * 48)
        nc.sync.wait_ge(st1, NT * 48)
        nc.scalar.wait_ge(st0, NT * 48)
        nc.scalar.wait_ge(st1, NT * 48)
    return nc


def kernel(x, height=48, width=48):
    from concourse.bass_utils import run_bass_kernel_spmd

    x = np.asarray(x)
    b, nh = x.shape[0], x.shape[1]
    xi = np.ascontiguousarray(x.reshape(b * nh, N, D))
    in_maps = [
        {"x": np.ascontiguousarray(xi[IMGS_PER_CORE * c: IMGS_PER_CORE * (c + 1)])}
        for c in range(N_CORES)
    ]
    if "nc" not in _CACHE:
        _CACHE["nc"] = _build_nc()
    res = run_bass_kernel_spmd(_CACHE["nc"], in_maps, core_ids=list(range(N_CORES)))
    y = np.stack([res.results[c]["out"] for c in range(N_CORES)])
    return y.reshape(b, nh, N, K, D).astype(np.float32, copy=False)


# revision 7
# speedup vs baseline: 1.5344x; 1.0902x over previous
"""LocalExpansion (7x7 unfold) Trainium2 Bass kernel, v2.

Full input x: [2, 8, 2304, 64] f32 (B=2, heads=8, N=48*48, D=64).
Full output:  [2, 8, 2304, 49, 64] f32 — out[b,h,y*W+x,i*7+j,:] =
x_img[b,h,y+i-3,x+j-3,:] with zero fill outside the 48x48 image.

Strategy (memory-regime, ~57.8 MB of HBM writes per core):
- batch*heads = 16 images, 2 per core across 8 NeuronCores.
- Partition map: image row y = 3g+r -> partition 4g+r (r in 0..2),
  image 0 in partitions 0-63, image 1 in 64-127. Skipping every 4th
  partition gives each of the 16 SDMA engines exactly 6 rows (the
  engine<->partition swizzle is quad-based), vs an 8/4 imbalance for
  a contiguous map. Skipped partitions stay zero; the fill op reads
  them but their U output is never stored.
- P[p] holds the 7 zero-padded image rows y(p)-3..y(p)+3 plus an
  always-zero row 7. Row interiors are loaded straight from HBM
  (x re-read ~7x: HBM reads are cheap, while SB2SB replication runs
  at half engine rate), pad columns come from two strip memsets, and
  boundary rows are zero-filled from row 7 on the gpsimd ring.
  Skipped/dup partitions are left uninitialized; the fill reads junk
  there but that U output is never stored.
- The unfold is materialized in SBUF, double-buffered in x-tiles of
  XT=3 columns: U[p][x - x0][i*7+j][d] = P[p][i][x+j][d]. The strided
  copy runs at 1x mode, so it is split: i-rows 0-3 on the Vector
  engine, 4-6 on the Scalar (ACT) engine.
- Output DMA per (tile, image): src and dst are both contiguous runs
  of 3*49*64 floats (37632 B descriptors, one per partition, written
  in ascending address order -> sequential HBM write streams), vs
  1792 B scattered descriptors for the direct sliding-window DMA.
- Image 0 DMAs ride the sync HWDGE ring / even SDMA engines; image 1
  rides the scalar ring / odd engines.
"""

import numpy as np

KH, KW = 7, 7
H, W, D = 48, 48, 64
K = KH * KW                 # 49
N = H * W                   # 2304
PR = (W + 6) * D            # 3456 floats per padded row
NR = (KH + 1) * PR          # 27648 floats of P per partition (row 7 = zeros)
XT = 3                      # x columns per tile
NT = W // XT                # 16 tiles
CH = XT * K * D             # 9408 floats per out chunk (37632 B)
FU = 2 * CH                 # 18816 floats of U per partition (2 bufs)
RB = W * K * D              # 150528 floats per output row
ND = N * D                  # floats per image input
NKD = N * K * D             # floats per image output
IMGS_PER_CORE = 2
N_CORES = 8

_CACHE = {}


def _build_nc():
    import concourse.bass as bass
    import concourse.mybir as mybir

    nc = bass.Bass(trn_type="TRN2")
    x = nc.dram_tensor("x", [IMGS_PER_CORE, N, D], mybir.dt.float32,
                       kind="ExternalInput")
    out = nc.dram_tensor("out", [IMGS_PER_CORE, N, K, D], mybir.dt.float32,
                         kind="ExternalOutput")

    with (
        nc.sbuf_tensor("P", [128, NR], mybir.dt.float32) as P,
        nc.sbuf_tensor("U", [128, FU], mybir.dt.float32) as U,
        nc.semaphore("ms") as ms,    # memsets (row-3 pads, zero row 7)
        nc.semaphore("ld") as ld,    # input loads
        nc.semaphore("zn") as zn,    # boundary-row zero fills
        nc.semaphore("fl") as fl,    # vector fills
        nc.semaphore("st0") as st0,  # out DMAs, sync ring (image 0)
        nc.semaphore("st1") as st1,  # out DMAs, scalar ring (image 1)
    ):
        # Memsets: pad columns of rows 0-6 (3-dim strips) and the
        # always-zero row 7 (zone source).
        nc.vector.memset(
            bass.AP(P, 0, [[NR, 128], [PR, KH], [1, 3 * D]]), 0.0
        ).then_inc(ms, 1)
        nc.vector.memset(
            bass.AP(P, 3 * D + W * D, [[NR, 128], [PR, KH], [1, 3 * D]]), 0.0
        ).then_inc(ms, 1)
        nc.vector.memset(
            bass.AP(P, KH * PR, [[NR, 128], [1, PR]]), 0.0
        ).then_inc(ms, 1)

        # P row interiors loaded straight from HBM: P[4g+r][i] = x row
        # 3g+r+i-3 (x re-read ~7x; HBM reads are cheap vs SB2SB
        # replication, which runs at half engine rate and serializes
        # ~21 DMAs/ring). Three DMAs per (image, r): bulk g in [1,15)
        # with all 7 rows valid, plus clamped g=0 / g=15 edges. No
        # dependencies - issued at t=0 (disjoint bytes from the
        # memsets; only the FIRST dim of an SBUF-side AP may step
        # partitions, inner dims stay within the partition).
        for im, ring in ((0, nc.sync), (1, nc.scalar)):
            for r in range(3):
                base = (64 * im + r) * NR
                ring.dma_start(
                    out=bass.AP(P, base + 4 * NR + 3 * D,
                                [[4 * NR, 14], [PR, KH], [1, W * D]]),
                    in_=bass.AP(x, im * ND + r * W * D,
                                [[3 * W * D, 14], [W * D, KH], [1, W * D]]),
                ).then_inc(ld, 16)
                ring.dma_start(  # g=0: rows i in [3-r, 7)
                    out=bass.AP(P, base + (3 - r) * PR + 3 * D,
                                [[NR, 1], [PR, 4 + r], [1, W * D]]),
                    in_=bass.AP(x, im * ND,
                                [[(4 + r) * W * D, 1], [W * D, 4 + r], [1, W * D]]),
                ).then_inc(ld, 16)
                ring.dma_start(  # g=15: rows i in [0, 6-r)
                    out=bass.AP(P, base + 60 * NR + 3 * D,
                                [[NR, 1], [PR, 6 - r], [1, W * D]]),
                    in_=bass.AP(x, im * ND + (42 + r) * W * D,
                                [[(6 - r) * W * D, 1], [W * D, 6 - r], [1, W * D]]),
                ).then_inc(ld, 16)
        n_ld = 18

        # Boundary rows (y+i-3 outside [0,48)) zero-filled from row 7 of
        # the same partition, on the gpsimd (SWDGE) ring, in parallel
        # with the loads. They only gate the fills (zn).
        zones = [(0, 0, 3), (0, 1, 2), (0, 2, 1),
                 (60, 6, 3), (61, 5, 2), (62, 4, 1)]
        nc.gpsimd.wait_ge(ms, 3)
        for im in (0, 1):
            for pb, row, cnt in zones:
                nc.gpsimd.dma_start(
                    out=bass.AP(P, (64 * im + pb) * NR + row * PR,
                                [[NR, cnt], [1, PR]]),
                    in_=bass.AP(P, (64 * im + pb) * NR + KH * PR,
                                [[NR, cnt], [1, PR]]),
                ).then_inc(zn, 16)
        n_zones = 12 * 16

        # Fills, double-buffered in x-tiles: U[p][x-x0][k][d] =
        # P[p][i][x+j][d] for k = i*7+j. The strided copy runs at 1x DVE
        # mode (~9.9us/tile for all 7 i-rows), which would tie the out
        # DMAs (~10.1us/tile); split i-rows 0-3 on vector and 4-6 on the
        # scalar (ACT) engine so the fill stays off the critical path.
        # (GpSimd takes 15us for the 3-row part - too slow.)
        splits = ((nc.vector, 0, 4), (nc.scalar, 4, 3))
        fpt = len(splits)
        for eng, _, _ in splits:
            eng.wait_ge(ld, n_ld * 16)
            eng.wait_ge(ms, 3)
            eng.wait_ge(zn, n_zones)

        def fill(eng, t, i0, ni):
            buf = t % 2
            if t >= 2:
                eng.wait_ge(st0, 48 * (t - 1))
                eng.wait_ge(st1, 48 * (t - 1))
            dst = bass.AP(U, buf * CH + i0 * KW * D,
                          [[FU, 127], [K * D, XT], [KW * D, ni], [1, KW * D]])
            src = bass.AP(P, t * XT * D + i0 * PR,
                          [[NR, 127], [D, XT], [PR, ni], [1, KW * D]])
            op = eng.tensor_copy(dst, src) if eng is nc.vector \
                else eng.copy(dst, src)
            op.then_inc(fl, 1)

        def store(t, im, ring, sem):
            buf = t % 2
            ring.wait_ge(fl, fpt * (t + 1))
            for r in range(3):
                ring.dma_start(
                    out=bass.AP(out, im * NKD + r * RB + t * CH,
                                [[3 * RB, 16], [1, CH]]),
                    in_=bass.AP(U, (64 * im + r) * FU + buf * CH,
                                [[4 * FU, 16], [1, CH]]),
                ).then_inc(sem, 16)

        # Vector: pure fill stream. Scalar: fill-part then its ring's
        # out-DMA issues, interleaved per tile (all-fills-then-all-
        # stores would deadlock the scalar sequencer on st1). Sync
        # ring: pure store stream.
        for t in range(NT):
            fill(nc.vector, t, 0, 4)
        for t in range(NT):
            fill(nc.scalar, t, 4, 3)
            store(t, 1, nc.scalar, st1)
        for t in range(NT):
            store(t, 0, nc.sync, st0)

        nc.sync.wait_ge(st0, NT * 48)
        nc.sync.wait_ge(st1, NT * 48)
        nc.scalar.wait_ge(st0, NT * 48)
        nc.scalar.wait_ge(st1, NT * 48)
    return nc


def kernel(x, height=48, width=48):
    from concourse.bass_utils import run_bass_kernel_spmd

    x = np.asarray(x)
    b, nh = x.shape[0], x.shape[1]
    xi = np.ascontiguousarray(x.reshape(b * nh, N, D))
    in_maps = [
        {"x": np.ascontiguousarray(xi[IMGS_PER_CORE * c: IMGS_PER_CORE * (c + 1)])}
        for c in range(N_CORES)
    ]
    if "nc" not in _CACHE:
        _CACHE["nc"] = _build_nc()
    res = run_bass_kernel_spmd(_CACHE["nc"], in_maps, core_ids=list(range(N_CORES)))
    y = np.stack([res.results[c]["out"] for c in range(N_CORES)])
    return y.reshape(b, nh, N, K, D).astype(np.float32, copy=False)
